# revision 1
# baseline (speedup 1.0000x reference)
"""BFP (block-floating-point) activation quantization on 8 Trainium2 NeuronCores.

Reference semantics (for mantissa_bits=3, blk=32, x: [32, 256, 56, 56] f32):
  per block of 32 consecutive channels (per n, h, w):
    maxabs = max|x|;  e = floor(log2(maxabs));  scale = 2^(e-2)
    out = clip(round_half_even(x/scale), -4, 3) * scale   (0 where maxabs==0)

Exact-math implementation used on device (all f32-exact, no transcendentals):
    M  = 2^e   (bit-mask the exponent field of maxabs -> exact)
    R  = 2^-e  (integer 0x7F000000 - M_bits -> exact)
    u  = x * R                      (exact power-of-two scale, u in (-2, 2))
    v  = min(max(u, -1.0), 0.75)    (pre-clip; equivalent to post-round clip)
    w  = (v + 1.5*2^21) - 1.5*2^21  (magic-number round-to-nearest-even to 1/4)
    out = w * M                     (exact)

Sharding: pure data-parallel, 4 images per core (batch 32 / 8 cores).
Per core the data is [4, 256, 3136]; images are processed in pairs so the
flattened free axis 2*3136 = 6272 is a multiple of 128 (needed for the
128x128 PE transposes).
"""

import os
import sys

sys.path.insert(0, "/opt/trn_rl_repo")

import numpy as np

import concourse.bass as bass
import concourse.bacc as bacc
import concourse.tile as tile
from concourse import masks, mybir
from concourse import bass_utils

F32 = mybir.dt.float32
I32 = mybir.dt.int32

N_CORES = 8
N, C, H, W = 32, 256, 56, 56
SP = H * W               # 3136
NPC = N // N_CORES       # 4 images per core
PAIR_F = 2 * SP          # 6272 free elems per (pair, 128-ch half)
STRIP = 896              # 7 strips of 896 = 6272; 896 = 7 * 128
NSTRIP = PAIR_F // STRIP # 7
NBLK = STRIP // 128      # 7 transpose blocks per strip

MAGIC = 3145728.0        # 1.5 * 2^21 : rounds to multiples of 1/4 in f32
EXP_MASK = 0x7F800000
RECIP_C = 0x7F000000     # bits(2^-e) = RECIP_C - bits(2^e)


def bfp_body(tc: tile.TileContext, x: bass.AP, y: bass.AP):
    nc = tc.nc

    const_pool = tc.alloc_tile_pool(name="consts", bufs=1)
    ident = const_pool.tile([128, 128], F32)
    masks.make_identity(nc, ident[:])
    mask_c = const_pool.tile([128, 1], I32)
    nc.vector.memset(mask_c[:], EXP_MASK)
    recip_c = const_pool.tile([128, 1], I32)
    nc.vector.memset(recip_c[:], RECIP_C)

    slab_pool = tc.alloc_tile_pool(name="slabs", bufs=2)
    strip_pool = tc.alloc_tile_pool(name="strips", bufs=4)
    small_pool = tc.alloc_tile_pool(name="small", bufs=4)
    psum_pool = tc.alloc_tile_pool(name="psum", bufs=2, space="PSUM")

    def bc(t, dt):
        return (
            t[:]
            .bitcast(dt)
            .rearrange("p (j b) -> p j b", j=NBLK)
            .unsqueeze(3)
            .broadcast_to([128, NBLK, 4, 32])
        )

    def front(x_sb, k):
        """PE transposes strip k, ACT copies PSUM->SBUF, DVE computes
        per-block maxabs and the exact 2^e / 2^-e tiles."""
        xT_ps = psum_pool.tile([128, STRIP], F32, tag="xT")
        for j in range(NBLK):
            col = k * STRIP + j * 128
            nc.tensor.transpose(
                xT_ps[:, j * 128 : j * 128 + 128], x_sb[:, col : col + 128],
                ident[:],
            )
        xT_sb = strip_pool.tile([128, STRIP], F32, tag="xT_sb")
        nc.scalar.copy(xT_sb[:], xT_ps[:])

        mx = small_pool.tile([128, NBLK * 4], F32, tag="mx")
        nc.vector.tensor_reduce(
            mx[:].rearrange("p (j b) -> p j b", j=NBLK),
            xT_sb[:].rearrange("p (j b c) -> p j b c", j=NBLK, b=4),
            axis=mybir.AxisListType.X,
            op=mybir.AluOpType.max,
            apply_absolute_value=True,
        )
        mb = small_pool.tile([128, NBLK * 4], I32, tag="mb")
        nc.vector.tensor_tensor(
            mb[:], mx[:].bitcast(I32),
            mask_c[:].broadcast_to([128, NBLK * 4]),
            op=mybir.AluOpType.bitwise_and,
        )
        rb = small_pool.tile([128, NBLK * 4], I32, tag="rb")
        nc.vector.tensor_tensor(
            rb[:], recip_c[:].broadcast_to([128, NBLK * 4]), mb[:],
            op=mybir.AluOpType.subtract,
        )
        return xT_sb, mb, rb

    def quant(st, k):
        """u = x*2^-e; v = clip(u); w = magic-round(v); o = w*2^e.
        Whole chain on one engine, alternating GPSIMD/DVE per strip."""
        xT_sb, mb, rb = st
        eng = nc.gpsimd if (k % 2 == 0) else nc.vector
        x4 = xT_sb[:].rearrange("p (j b c) -> p j b c", j=NBLK, b=4)
        u = strip_pool.tile([128, STRIP], F32, tag="u")
        eng.tensor_tensor(
            u[:].rearrange("p (j b c) -> p j b c", j=NBLK, b=4),
            x4, bc(rb, F32), op=mybir.AluOpType.mult,
        )
        v = strip_pool.tile([128, STRIP], F32, tag="v")
        eng.tensor_scalar(
            v[:], u[:], -1.0, 0.75,
            op0=mybir.AluOpType.max, op1=mybir.AluOpType.min,
        )
        w = strip_pool.tile([128, STRIP], F32, tag="w")
        eng.tensor_scalar(
            w[:], v[:], MAGIC, MAGIC,
            op0=mybir.AluOpType.add, op1=mybir.AluOpType.subtract,
        )
        o = strip_pool.tile([128, STRIP], F32, tag="o")
        eng.tensor_tensor(
            o[:].rearrange("p (j b c) -> p j b c", j=NBLK, b=4),
            w[:].rearrange("p (j b c) -> p j b c", j=NBLK, b=4),
            bc(mb, F32), op=mybir.AluOpType.mult,
        )
        return o

    def back(o, out_sb, k):
        """PE back-transposes strip k, copy PSUM->out slab."""
        wT_ps = psum_pool.tile([128, STRIP], F32, tag="wT")
        for j in range(NBLK):
            nc.tensor.transpose(
                wT_ps[:, j * 128 : j * 128 + 128],
                o[:, j * 128 : j * 128 + 128], ident[:],
            )
        if k % 2 == 0:
            nc.scalar.copy(out_sb[:, k * STRIP : (k + 1) * STRIP], wT_ps[:])
        else:
            nc.vector.tensor_copy(
                out_sb[:, k * STRIP : (k + 1) * STRIP], wT_ps[:]
            )

    for rep in range(int(os.environ.get("BFP_ITERS", "1"))):
      for pair in range(NPC // 2):
        for chh in range(C // 128):
              x_sb = slab_pool.tile([128, PAIR_F], F32, tag="x_sb")
              out_sb = slab_pool.tile([128, PAIR_F], F32, tag="out_sb")
              # two half-slab DMAs (one per image): first strips start
              # after 1.6MB lands instead of the full 3.2MB slab
              for h in range(2):
                  nc.sync.dma_start(
                      out=x_sb[:, h * SP : (h + 1) * SP],
                      in_=x[2 * pair + h, 128 * chh : 128 * chh + 128, :],
                  )

              # 3-stage skewed software pipeline: front(k) | quant(k-1) |
              # back(k-2). Keeps PE's forward transposes ahead of its back
              # transposes in program order so the in-order engines never
              # head-of-line block on the strip currently being quantized.
              st = {}
              oo = {}
              for k in range(NSTRIP + 2):
                  if k < NSTRIP:
                      st[k] = front(x_sb, k)
                  if 0 <= k - 1 < NSTRIP:
                      oo[k - 1] = quant(st.pop(k - 1), k - 1)
                  if k - 2 >= 0:
                      back(oo.pop(k - 2), out_sb, k - 2)

              # outputs on the second HWDGE ring (ACT-triggered) so input and
              # output transfers overlap instead of serializing in one FIFO
              for h in range(2):
                  nc.scalar.dma_start(
                      out=y[2 * pair + h, 128 * chh : 128 * chh + 128, :],
                      in_=out_sb[:, h * SP : (h + 1) * SP],
                  )

    for p in (psum_pool, small_pool, strip_pool, slab_pool, const_pool):
        p.release()


_CACHED = None


def _build():
    global _CACHED
    if _CACHED is None:
        nc = bacc.Bacc("TRN2", target_bir_lowering=False, debug=False)
        x = nc.dram_tensor("x", [NPC, C, SP], F32, kind="ExternalInput")
        y = nc.dram_tensor("y", [NPC, C, SP], F32, kind="ExternalOutput")
        with tile.TileContext(nc) as tc:
            bfp_body(tc, x[:], y[:])
        nc.compile()
        _CACHED = nc
    return _CACHED


def kernel(activations, mantissa_bits, blk, _trace=False, _tmpdir=None):
    mb = int(np.asarray(mantissa_bits))
    b = int(np.asarray(blk))
    assert mb == 3 and b == 32, (mb, b)
    x = np.ascontiguousarray(np.asarray(activations, dtype=np.float32))
    assert x.shape == (N, C, H, W), x.shape

    xs = x.reshape(N_CORES, NPC, C, SP)
    in_maps = [{"x": xs[k]} for k in range(N_CORES)]
    nc = _build()
    res = bass_utils.run_bass_kernel_spmd(
        nc, in_maps, core_ids=list(range(N_CORES)), trace=_trace, tmpdir=_tmpdir
    )
    outs = [np.asarray(res.results[k]["y"]) for k in range(N_CORES)]
    out = np.stack(outs, axis=0).reshape(N, C, H, W)
    if _trace:
        return out, res
    return out



# revision 16
# speedup vs baseline: 1.7443x; 1.7443x over previous
"""BFP (block-floating-point) activation quantization on 8 Trainium2 NeuronCores.

Reference semantics (for mantissa_bits=3, blk=32, x: [32, 256, 56, 56] f32):
  per block of 32 consecutive channels (per n, h, w):
    maxabs = max|x|;  e = floor(log2(maxabs));  scale = 2^(e-2)
    out = clip(round_half_even(x/scale), -4, 3) * scale   (0 where maxabs==0)

Exact-math implementation used on device (all f32-exact, no transcendentals):
    M  = 2^e   (bit-mask the exponent field of maxabs -> exact)
    R  = 2^-e  (integer 0x7F000000 - M_bits -> exact)
    u  = x * R                      (exact power-of-two scale, u in (-2, 2))
    w  = min(max(u, -1.0), 0.75)    (pre-clip; equivalent to post-round clip)
    v  = (w + 1.5*2^21) - 1.5*2^21  (magic-number round-to-nearest-even to 1/4)
    out = v * M                     (exact)

The clip/round/scale tail (w, v, out) runs as ONE custom DVE instruction
(BFP_ROUND_SCALE_ANT below); the u multiply and the tiny exponent-mask ops
run on GPSIMD; the per-block absmax reduce runs on DVE; PE does the layout
transposes (f32r tagged: 1.5 cycles/row); ACT does the PSUM->SBUF copies.
Every engine's total busy time sits under the DMA roofline (in+out HBM
traffic at 360 GB/s/core), and a 5-deep cross-slab software pipeline
(front | quant | back, two strips of skew between stages) keeps the DMA
engines saturated.

Sharding: pure data-parallel, 4 images per core (batch 32 / 8 cores).
Per core the data is [4, 256, 3136]; images are processed in pairs so the
flattened free axis 2*3136 = 6272 is a multiple of 128 (needed for the
128x128 PE transposes).
"""

import os
import sys

sys.path.insert(0, "/opt/trn_rl_repo")

import numpy as np

import concourse.bass as bass
import concourse.bacc as bacc
import concourse.tile as tile
from concourse import masks, mybir
from concourse import bass_utils
import concourse.dve_ops as dve_ops
from concourse.dve_spec import AluOp, Bin, Spec, Src0, Src1, C0, C1, Zero, lower, maxx, minn
from concourse.dve_uop import DveOpSpec

F32 = mybir.dt.float32
F32R = mybir.dt.float32r
I32 = mybir.dt.int32

N_CORES = 8
N, C, H, W = 32, 256, 56, 56
SP = H * W               # 3136
NPC = N // N_CORES       # 4 images per core
PAIR_F = 2 * SP          # 6272 free elems per (pair, 128-ch half)
STRIP = 896              # 7 strips of 896 = 6272; 896 = 7 * 128
NSTRIP = PAIR_F // STRIP # 7
NBLK = STRIP // 128      # 7 transpose tiles per strip
NGRP = NBLK * 4          # 28 (tile, block) groups per strip
NSLAB = (NPC // 2) * (C // 128)  # 4 slabs per core
NG = NSLAB * NSTRIP      # 28 strips per core

MAGIC = 3145728.0        # 1.5 * 2^21 : rounds to multiples of 1/4 in f32
QMAXW = 0.75             # upper clip in w units (= qmax/4)
EXP_MASK = 0x7F800000
RECIP_C = 0x7F000000     # bits(2^-e) = RECIP_C - bits(2^e)


# ---------------------------------------------------------------------------
# Custom DVE op: out = ((min(max(in0, -1), C0) + C1) - C1) * in1
#   C0 = 0.75, C1 = MAGIC;  -1 is synthesized as Zero - (Zero >= Zero).
# One DVE pass for clip + round-to-quarters + scale-by-2^e.
# ---------------------------------------------------------------------------

def _bfp_ref(in0, in1, s0, s1, imm2):
    f32 = np.float32
    w = np.maximum(np.minimum(in0.astype(f32), f32(s0)), f32(-1.0)).astype(f32)
    t = (w + f32(s1)).astype(f32)
    v = (t - f32(s1)).astype(f32)
    return (v * in1.astype(f32)).astype(f32)


def _make_bfp_op():
    one = Bin(AluOp.IS_GE, Zero, Zero)
    negone = Bin(AluOp.SUBTRACT, Zero, one)
    body = Bin(
        AluOp.MULTIPLY,
        Bin(
            AluOp.SUBTRACT,
            Bin(AluOp.ADD, maxx(minn(Src0, C0), negone), C1),
            C1,
        ),
        Src1,
    )
    spec = Spec(body=body, reference=_bfp_ref)
    name = "BFP_ROUND_SCALE_ANT"
    if any(op.name == name for op in dve_ops.OPS):
        return next(op for op in dve_ops.OPS if op.name == name)
    row = max(dve_ops._SUB_OPCODE_FOR_NAME.values()) + 1
    assert row < 0x20
    dve_ops._SUB_OPCODE_FOR_NAME[name] = row
    shas = {}
    for ver in ("v3", "v4"):
        try:
            s = DveOpSpec(name=name, opcode=row, uops=lower(spec, ver=ver), rd1_en=True)
            shas[ver] = s.sha(ver)
        except Exception:
            pass
    op = dve_ops.DveOp(name=name, spec=spec, subdim=False, uops_sha=shas)
    dve_ops.OPS.append(op)
    dve_ops.CUSTOM_DVE_SPECS[name] = spec
    return op


BFP_OP = _make_bfp_op()


def bfp_body(tc: tile.TileContext, x: bass.AP, y: bass.AP):
    nc = tc.nc

    const_pool = tc.alloc_tile_pool(name="consts", bufs=1)
    ident = const_pool.tile([128, 128], F32)
    masks.make_identity(nc, ident[:])
    mask_c = const_pool.tile([128, 1], I32)
    nc.vector.memset(mask_c[:], EXP_MASK)
    recip_c = const_pool.tile([128, 1], I32)
    nc.vector.memset(recip_c[:], RECIP_C)

    strip_pool = tc.alloc_tile_pool(name="strips", bufs=2)
    small_pool = tc.alloc_tile_pool(name="small", bufs=2)
    psum_f = tc.alloc_tile_pool(name="psum_f", bufs=2, space="PSUM")
    psum_b = tc.alloc_tile_pool(name="psum_b", bufs=2, space="PSUM")

    slabs = [(p, h) for p in range(NPC // 2) for h in range(C // 128)]

    def strip_spans(g):
        """DRAM (img, sp) spans covered by global strip g: 1-2 contiguous runs."""
        s, k = divmod(g, NSTRIP)
        pair, chh = slabs[s]
        lo, hi = k * STRIP, (k + 1) * STRIP
        out = []
        for h in range(2):
            a, b = max(lo, h * SP), min(hi, (h + 1) * SP)
            if a < b:
                out.append((2 * pair + h, 128 * chh, a - h * SP, b - h * SP, a - lo))
        return out

    # -- pipeline stages (one strip each; every cross-engine data edge is --
    # -- >=1 pipeline step old when consumed, so no engine head-of-line  --
    # -- blocks on another engine's current-step work                    --

    def stage_in(g):        # SP: per-strip input DMA (1-2 transfers)
        xs = strip_pool.tile([128, STRIP], F32, tag="xs", name="xs", bufs=6)
        for img, ch, a, b, off in strip_spans(g):
            nc.sync.dma_start(
                out=xs[:, off : off + (b - a)],
                in_=x[img, ch : ch + 128, a:b],
            )
        return xs

    def stage_fwd(xs):      # PE: 7 transposes (f32r: 1.5 cyc/row)
        xT_ps = psum_f.tile([128, STRIP], F32, tag="xT", name="xT_ps")
        for j in range(NBLK):
            nc.tensor.transpose(
                xT_ps[:, j * 128 : j * 128 + 128],
                xs[:, j * 128 : j * 128 + 128],
                ident[:],
            )
        return xT_ps

    def stage_copyin(xT_ps):  # ACT: PSUM -> SBUF
        xT_sb = strip_pool.tile([128, STRIP], F32, tag="xT_sb", name="xT_sb", bufs=4)
        nc.scalar.copy(xT_sb[:], xT_ps[:])
        return xT_sb

    def stage_reduce(xT_sb):  # DVE: per-block absmax
        mx = small_pool.tile([128, NGRP], F32, tag="mx", name="mx")
        nc.vector.tensor_reduce(
            mx[:].rearrange("p (g b) -> p g b", g=NGRP),
            xT_sb[:].rearrange("p (g c) -> p g c", g=NGRP),
            axis=mybir.AxisListType.X,
            op=mybir.AluOpType.max,
            apply_absolute_value=True,
        )
        return mx

    def stage_mask(mx):     # DVE: 2^e / 2^-e bit tiles (tiny; int ops are DVE-only)
        mb = small_pool.tile([128, NGRP], I32, tag="mb", name="mb", bufs=3)
        nc.vector.tensor_tensor(
            mb[:], mx[:].bitcast(I32),
            mask_c[:].broadcast_to([128, NGRP]),
            op=mybir.AluOpType.bitwise_and,
        )
        rb = small_pool.tile([128, NGRP], I32, tag="rb", name="rb")
        nc.vector.tensor_tensor(
            rb[:], recip_c[:].broadcast_to([128, NGRP]), mb[:],
            op=mybir.AluOpType.subtract,
        )
        return mb, rb

    def stage_u(xT_sb, rb):  # Pool: u = x * 2^-e (exact power-of-two scale)
        rb_b = (
            rb[:].bitcast(F32).rearrange("p g -> p g ()").broadcast_to([128, NGRP, 32])
        )
        u = strip_pool.tile([128, STRIP], F32, tag="u", name="u")
        nc.gpsimd.tensor_tensor(
            u[:].rearrange("p (g c) -> p g c", g=NGRP),
            xT_sb[:].rearrange("p (g c) -> p g c", g=NGRP),
            rb_b, op=mybir.AluOpType.mult,
        )
        return u

    def stage_custom(u, mb):  # DVE: clip/round/scale in one custom op
        mb_b = (
            mb[:].bitcast(F32).rearrange("p g -> p g ()").broadcast_to([128, NGRP, 32])
        )
        o = strip_pool.tile([128, STRIP], F32, tag="o", name="o")
        nc.vector._custom_dve(
            BFP_OP,
            out=o[:].rearrange("p (g c) -> p g c", g=NGRP),
            in0=u[:].rearrange("p (g c) -> p g c", g=NGRP),
            in1=mb_b,
            s0=QMAXW,
            s1=MAGIC,
        )
        return o

    def stage_back(o):      # PE: 7 back-transposes
        oT_ps = psum_b.tile([128, STRIP], F32, tag="oT", name="oT_ps")
        for j in range(NBLK):
            nc.tensor.transpose(
                oT_ps[:, j * 128 : j * 128 + 128],
                o[:, j * 128 : j * 128 + 128],
                ident[:],
            )
        return oT_ps

    def stage_out(oT_ps, g):  # ACT: PSUM -> SBUF, then straight out to DRAM
        ot = strip_pool.tile([128, STRIP], F32, tag="ot", name="ot", bufs=3)
        nc.scalar.copy(ot[:], oT_ps[:])
        for img, ch, a, b, off in strip_spans(g):
            nc.scalar.dma_start(
                out=y[img, ch : ch + 128, a:b],
                in_=ot[:, off : off + (b - a)],
            )

    # 8-stage software pipeline over all 28 strips.  Stage offsets:
    #   in(g+2) | fwd(g) | copyin(g-1) | reduce(g-2) | mask(g-3) | u(g-4)
    #   | custom(g-5) | back(g-6) | out(g-7)
    # Emission order is deepest-stage-first so each in-order engine's next
    # instruction only consumes results that are already >=1 step old.
    LEAD = 3  # input DMA runs this many steps ahead of fwd
    for rep in range(int(os.environ.get("BFP_ITERS", "1"))):
        xs_t = {}
        xT_ps_t = {}
        xT_sb_t = {}
        mx_t = {}
        mbrb_t = {}
        u_t = {}
        o_t = {}
        oT_t = {}
        for g in range(NG + 7):
            if 0 <= g - 7 < NG:
                stage_out(oT_t.pop(g - 7), g - 7)
            if 0 <= g - 6 < NG:
                oT_t[g - 6] = stage_back(o_t.pop(g - 6))
            if 0 <= g - 5 < NG:
                o_t[g - 5] = stage_custom(u_t.pop(g - 5), mbrb_t.pop(g - 5)[0])
            if 0 <= g - 4 < NG:
                u_t[g - 4] = stage_u(xT_sb_t.pop(g - 4), mbrb_t[g - 4][1])
            if 0 <= g - 3 < NG:
                mbrb_t[g - 3] = stage_mask(mx_t.pop(g - 3))
            if 0 <= g - 2 < NG:
                mx_t[g - 2] = stage_reduce(xT_sb_t[g - 2])
            if 0 <= g - 1 < NG:
                xT_sb_t[g - 1] = stage_copyin(xT_ps_t.pop(g - 1))
            if g < NG:
                lo = g if g > 0 else 0
                hi = min(g + LEAD + 1, NG) if g > 0 else min(LEAD + 1, NG)
                for nxt in range(lo, hi):
                    if nxt not in xs_t:
                        xs_t[nxt] = stage_in(nxt)
                xT_ps_t[g] = stage_fwd(xs_t.pop(g))

    for p in (psum_b, psum_f, small_pool, strip_pool, const_pool):
        p.release()


_CACHED = None


def _build():
    global _CACHED
    if _CACHED is None:
        nc = bacc.Bacc("TRN2", target_bir_lowering=False, debug=False)
        x = nc.dram_tensor("x", [NPC, C, SP], F32, kind="ExternalInput")
        y = nc.dram_tensor("y", [NPC, C, SP], F32, kind="ExternalOutput")
        with tile.TileContext(nc) as tc:
            bfp_body(tc, x[:], y[:])
        nc.compile()
        _CACHED = nc
    return _CACHED


def kernel(activations, mantissa_bits, blk, _trace=False, _tmpdir=None):
    mb = int(np.asarray(mantissa_bits))
    b = int(np.asarray(blk))
    assert mb == 3 and b == 32, (mb, b)
    x = np.ascontiguousarray(np.asarray(activations, dtype=np.float32))
    assert x.shape == (N, C, H, W), x.shape

    xs = x.reshape(N_CORES, NPC, C, SP)
    in_maps = [{"x": xs[k]} for k in range(N_CORES)]
    nc = _build()
    res = bass_utils.run_bass_kernel_spmd(
        nc, in_maps, core_ids=list(range(N_CORES)), trace=_trace, tmpdir=_tmpdir
    )
    outs = [np.asarray(res.results[k]["y"]) for k in range(N_CORES)]
    out = np.stack(outs, axis=0).reshape(N, C, H, W)
    if _trace:
        return out, res
    return out


# revision 32
# speedup vs baseline: 1.7583x; 1.0080x over previous
"""BFP (block-floating-point) activation quantization on 8 Trainium2 NeuronCores.

Reference semantics (for mantissa_bits=3, blk=32, x: [32, 256, 56, 56] f32):
  per block of 32 consecutive channels (per n, h, w):
    maxabs = max|x|;  e = floor(log2(maxabs));  scale = 2^(e-2)
    out = clip(round_half_even(x/scale), -4, 3) * scale   (0 where maxabs==0)

Exact-math implementation used on device (all f32-exact, no transcendentals):
    M  = 2^e   (bit-mask the exponent field of maxabs -> exact)
    R  = 2^-e  (integer 0x7F000000 - M_bits -> exact)
    u  = x * R                      (exact power-of-two scale, u in (-2, 2))
    w  = min(max(u, -1.0), 0.75)    (pre-clip; equivalent to post-round clip)
    v  = (w + 1.5*2^21) - 1.5*2^21  (magic-number round-to-nearest-even to 1/4)
    out = v * M                     (exact)

The clip/round/scale tail (w, v, out) runs as ONE custom DVE instruction
(BFP_ROUND_SCALE_ANT below); the u multiply and the tiny exponent-mask ops
run on GPSIMD; the per-block absmax reduce runs on DVE; PE does the layout
transposes (f32r tagged: 1.5 cycles/row); ACT does the PSUM->SBUF copies.
Every engine's total busy time sits under the DMA roofline (in+out HBM
traffic at 360 GB/s/core), and a 5-deep cross-slab software pipeline
(front | quant | back, two strips of skew between stages) keeps the DMA
engines saturated.

Sharding: pure data-parallel, 4 images per core (batch 32 / 8 cores).
Per core the data is [4, 256, 3136]; images are processed in pairs so the
flattened free axis 2*3136 = 6272 is a multiple of 128 (needed for the
128x128 PE transposes).
"""

import os
import sys

sys.path.insert(0, "/opt/trn_rl_repo")

import numpy as np

import concourse.bass as bass
import concourse.bacc as bacc
import concourse.tile as tile
from concourse import masks, mybir
from concourse import bass_utils
import concourse.dve_ops as dve_ops
from concourse.dve_spec import AluOp, Bin, Spec, Src0, Src1, C0, C1, Zero, lower, maxx, minn
from concourse.dve_uop import DveOpSpec

F32 = mybir.dt.float32
F32R = mybir.dt.float32r
I32 = mybir.dt.int32

N_CORES = 8
N, C, H, W = 32, 256, 56, 56
SP = H * W               # 3136
NPC = N // N_CORES       # 4 images per core
PAIR_F = 2 * SP          # 6272 free elems per (pair, 128-ch half)
STRIP = 896              # 7 strips of 896 = 6272; 896 = 7 * 128
NSTRIP = PAIR_F // STRIP # 7
NBLK = STRIP // 128      # 7 transpose tiles per strip
NGRP = NBLK * 4          # 28 (tile, block) groups per strip
NSLAB = (NPC // 2) * (C // 128)  # 4 slabs per core
NG = NSLAB * NSTRIP      # 28 strips per core

MAGIC = 3145728.0        # 1.5 * 2^21 : rounds to multiples of 1/4 in f32
QMAXW = 0.75             # upper clip in w units (= qmax/4)
EXP_MASK = 0x7F800000
RECIP_C = 0x7F000000     # bits(2^-e) = RECIP_C - bits(2^e)


# ---------------------------------------------------------------------------
# Custom DVE op: out = ((min(max(in0, -1), C0) + C1) - C1) * in1
#   C0 = 0.75, C1 = MAGIC;  -1 is synthesized as Zero - (Zero >= Zero).
# One DVE pass for clip + round-to-quarters + scale-by-2^e.
# ---------------------------------------------------------------------------

def _bfp_ref(in0, in1, s0, s1, imm2):
    f32 = np.float32
    w = np.maximum(np.minimum(in0.astype(f32), f32(s0)), f32(-1.0)).astype(f32)
    t = (w + f32(s1)).astype(f32)
    v = (t - f32(s1)).astype(f32)
    return (v * in1.astype(f32)).astype(f32)


def _make_bfp_op():
    one = Bin(AluOp.IS_GE, Zero, Zero)
    negone = Bin(AluOp.SUBTRACT, Zero, one)
    body = Bin(
        AluOp.MULTIPLY,
        Bin(
            AluOp.SUBTRACT,
            Bin(AluOp.ADD, maxx(minn(Src0, C0), negone), C1),
            C1,
        ),
        Src1,
    )
    spec = Spec(body=body, reference=_bfp_ref)
    name = "BFP_ROUND_SCALE_ANT"
    if any(op.name == name for op in dve_ops.OPS):
        return next(op for op in dve_ops.OPS if op.name == name)
    row = max(dve_ops._SUB_OPCODE_FOR_NAME.values()) + 1
    assert row < 0x20
    dve_ops._SUB_OPCODE_FOR_NAME[name] = row
    shas = {}
    for ver in ("v3", "v4"):
        try:
            s = DveOpSpec(name=name, opcode=row, uops=lower(spec, ver=ver), rd1_en=True)
            shas[ver] = s.sha(ver)
        except Exception:
            pass
    op = dve_ops.DveOp(name=name, spec=spec, subdim=False, uops_sha=shas)
    dve_ops.OPS.append(op)
    dve_ops.CUSTOM_DVE_SPECS[name] = spec
    return op


BFP_OP = _make_bfp_op()


def bfp_body(tc: tile.TileContext, x: bass.AP, y: bass.AP):
    nc = tc.nc

    const_pool = tc.alloc_tile_pool(name="consts", bufs=1)
    ident = const_pool.tile([128, 128], F32)
    masks.make_identity(nc, ident[:])
    mask_c = const_pool.tile([128, 1], I32)
    nc.vector.memset(mask_c[:], EXP_MASK)
    recip_c = const_pool.tile([128, 1], I32)
    nc.vector.memset(recip_c[:], RECIP_C)

    strip_pool = tc.alloc_tile_pool(name="strips", bufs=2)
    small_pool = tc.alloc_tile_pool(name="small", bufs=2)
    psum_f = tc.alloc_tile_pool(name="psum_f", bufs=2, space="PSUM")
    psum_b = tc.alloc_tile_pool(name="psum_b", bufs=2, space="PSUM")

    slabs = [(p, h) for p in range(NPC // 2) for h in range(C // 128)]

    def strip_spans(g):
        """DRAM (img, sp) spans covered by global strip g: 1-2 contiguous runs."""
        s, k = divmod(g, NSTRIP)
        pair, chh = slabs[s]
        lo, hi = k * STRIP, (k + 1) * STRIP
        out = []
        for h in range(2):
            a, b = max(lo, h * SP), min(hi, (h + 1) * SP)
            if a < b:
                out.append((2 * pair + h, 128 * chh, a - h * SP, b - h * SP, a - lo))
        return out

    # -- pipeline stages (one strip each; every cross-engine data edge is --
    # -- >=1 pipeline step old when consumed, so no engine head-of-line  --
    # -- blocks on another engine's current-step work                    --

    def stage_in(g):        # SP: per-strip input DMA (1-2 transfers)
        xs = strip_pool.tile([128, STRIP], F32, tag="xs", name="xs", bufs=11)
        for img, ch, a, b, off in strip_spans(g):
            nc.sync.dma_start(
                out=xs[:, off : off + (b - a)],
                in_=x[img, ch : ch + 128, a:b],
            )
        return xs

    def stage_fwd(xs):      # PE: 7 transposes (f32r: 1.5 cyc/row)
        xT_ps = psum_f.tile([128, STRIP], F32, tag="xT", name="xT_ps")
        for j in range(NBLK):
            nc.tensor.transpose(
                xT_ps[:, j * 128 : j * 128 + 128],
                xs[:, j * 128 : j * 128 + 128],
                ident[:],
            )
        return xT_ps

    def stage_copyin(xT_ps):  # ACT: PSUM -> SBUF
        xT_sb = strip_pool.tile([128, STRIP], F32, tag="xT_sb", name="xT_sb", bufs=4)
        nc.scalar.copy(xT_sb[:], xT_ps[:])
        return xT_sb

    def stage_reduce(xT_sb):  # DVE: per-block absmax
        mx = small_pool.tile([128, NGRP], F32, tag="mx", name="mx")
        nc.vector.tensor_reduce(
            mx[:].rearrange("p (g b) -> p g b", g=NGRP),
            xT_sb[:].rearrange("p (g c) -> p g c", g=NGRP),
            axis=mybir.AxisListType.X,
            op=mybir.AluOpType.max,
            apply_absolute_value=True,
        )
        return mx

    def stage_mask(mx):     # 2^e (DVE: bitwise int ops are DVE-only) and
        mb = small_pool.tile([128, NGRP], I32, tag="mb", name="mb", bufs=3)
        nc.vector.tensor_tensor(
            mb[:], mx[:].bitcast(I32),
            mask_c[:].broadcast_to([128, NGRP]),
            op=mybir.AluOpType.bitwise_and,
        )
        rb = small_pool.tile([128, NGRP], I32, tag="rb", name="rb")
        nc.vector.tensor_tensor(
            rb[:], recip_c[:].broadcast_to([128, NGRP]), mb[:],
            op=mybir.AluOpType.subtract,
        )
        return mb, rb

    def stage_u(xT_sb, rb):  # Pool: u = x * 2^-e (exact power-of-two scale)
        rb_b = (
            rb[:].bitcast(F32).rearrange("p g -> p g ()").broadcast_to([128, NGRP, 32])
        )
        u = strip_pool.tile([128, STRIP], F32, tag="u", name="u")
        nc.gpsimd.tensor_tensor(
            u[:].rearrange("p (g c) -> p g c", g=NGRP),
            xT_sb[:].rearrange("p (g c) -> p g c", g=NGRP),
            rb_b, op=mybir.AluOpType.mult,
        )
        return u

    def stage_custom(u, mb):  # DVE: clip/round/scale in one custom op
        mb_b = (
            mb[:].bitcast(F32).rearrange("p g -> p g ()").broadcast_to([128, NGRP, 32])
        )
        o = strip_pool.tile([128, STRIP], F32, tag="o", name="o")
        nc.vector._custom_dve(
            BFP_OP,
            out=o[:].rearrange("p (g c) -> p g c", g=NGRP),
            in0=u[:].rearrange("p (g c) -> p g c", g=NGRP),
            in1=mb_b,
            s0=QMAXW,
            s1=MAGIC,
        )
        return o

    def stage_back(o):      # PE: 7 back-transposes
        oT_ps = psum_b.tile([128, STRIP], F32, tag="oT", name="oT_ps")
        for j in range(NBLK):
            nc.tensor.transpose(
                oT_ps[:, j * 128 : j * 128 + 128],
                o[:, j * 128 : j * 128 + 128],
                ident[:],
            )
        return oT_ps

    def stage_out(oT_ps, g):  # ACT: PSUM -> SBUF, then straight out to DRAM
        ot = strip_pool.tile([128, STRIP], F32, tag="ot", name="ot", bufs=3)
        nc.scalar.copy(ot[:], oT_ps[:])
        for img, ch, a, b, off in strip_spans(g):
            nc.sync.dma_start(
                out=y[img, ch : ch + 128, a:b],
                in_=ot[:, off : off + (b - a)],
            )

    # 8-stage software pipeline over all 28 strips.  Stage offsets:
    #   in(g+2) | fwd(g) | copyin(g-1) | reduce(g-2) | mask(g-3) | u(g-4)
    #   | custom(g-5) | back(g-6) | out(g-7)
    # Emission order is deepest-stage-first so each in-order engine's next
    # instruction only consumes results that are already >=1 step old.
    LEAD = 3   # steady-state input-DMA run-ahead (strips)
    PREFETCH = 10  # initial input-DMA burst before compute warms up
    for rep in range(int(os.environ.get("BFP_ITERS", "1"))):
        xs_t = {}
        xT_ps_t = {}
        xT_sb_t = {}
        mx_t = {}
        mbrb_t = {}
        u_t = {}
        o_t = {}
        oT_t = {}
        for g in range(NG + 7):
            if 0 <= g - 7 < NG:
                stage_out(oT_t.pop(g - 7), g - 7)
            if 0 <= g - 6 < NG:
                oT_t[g - 6] = stage_back(o_t.pop(g - 6))
            if 0 <= g - 5 < NG:
                o_t[g - 5] = stage_custom(u_t.pop(g - 5), mbrb_t.pop(g - 5)[0])
            if 0 <= g - 4 < NG:
                u_t[g - 4] = stage_u(xT_sb_t.pop(g - 4), mbrb_t[g - 4][1])
            if 0 <= g - 3 < NG:
                mbrb_t[g - 3] = stage_mask(mx_t.pop(g - 3))
            if 0 <= g - 2 < NG:
                mx_t[g - 2] = stage_reduce(xT_sb_t[g - 2])
            if 0 <= g - 1 < NG:
                xT_sb_t[g - 1] = stage_copyin(xT_ps_t.pop(g - 1))
            if g < NG:
                hi = min(g + LEAD + 1, NG) if g > 0 else min(PREFETCH, NG)
                for nxt in range(g, hi):
                    if nxt not in xs_t:
                        xs_t[nxt] = stage_in(nxt)
                xT_ps_t[g] = stage_fwd(xs_t.pop(g))

    for p in (psum_b, psum_f, small_pool, strip_pool, const_pool):
        p.release()


_CACHED = None


def _build():
    global _CACHED
    if _CACHED is None:
        nc = bacc.Bacc("TRN2", target_bir_lowering=False, debug=False)
        x = nc.dram_tensor("x", [NPC, C, SP], F32, kind="ExternalInput")
        y = nc.dram_tensor("y", [NPC, C, SP], F32, kind="ExternalOutput")
        with tile.TileContext(nc) as tc:
            bfp_body(tc, x[:], y[:])
        nc.compile()
        _CACHED = nc
    return _CACHED


def kernel(activations, mantissa_bits, blk, _trace=False, _tmpdir=None):
    mb = int(np.asarray(mantissa_bits))
    b = int(np.asarray(blk))
    assert mb == 3 and b == 32, (mb, b)
    x = np.ascontiguousarray(np.asarray(activations, dtype=np.float32))
    assert x.shape == (N, C, H, W), x.shape

    xs = x.reshape(N_CORES, NPC, C, SP)
    in_maps = [{"x": xs[k]} for k in range(N_CORES)]
    nc = _build()
    res = bass_utils.run_bass_kernel_spmd(
        nc, in_maps, core_ids=list(range(N_CORES)), trace=_trace, tmpdir=_tmpdir
    )
    outs = [np.asarray(res.results[k]["y"]) for k in range(N_CORES)]
    out = np.stack(outs, axis=0).reshape(N, C, H, W)
    if _trace:
        return out, res
    return out
